# revision 1
# baseline (speedup 1.0000x reference)
"""GCN encoder (2x GCNConv + mu/logvar heads) on 8 Trainium2 NeuronCores.

Strategy: shard destination nodes (and their incoming edges) across the 8
cores.  Each layer:
  1. dense matmul of the core's node shard by the (replicated) weight,
     pre-scaled by dinv = deg^-1/2   (fp32 on PE, activations PE-transposed)
  2. AllGather the scaled feature table so every core can gather any source row
  3. aggregation over the core's dest-sorted edges: dma_gather source rows
     (int16 windowed indices), build one-hot selection matrices with DVE
     is_equal against an iota row, and accumulate 128-edge x 128-dest
     selection matmuls (float32r) into a PSUM tile per 128-dest group
  4. epilogue on ACT: out = relu(dinv * psum) (+bias) for layer 1,
     out = dinv * psum (+bias) for layer 2.
mu/logvar heads are small dense matmuls on the aggregated shard.
"""

import sys

import numpy as np

try:
    import concourse.bass as bass  # noqa: F401
except ImportError:
    sys.path.insert(0, "/opt/trn_rl_repo")

import concourse.bass as bass
import concourse.mybir as mybir
import concourse.tile as tile
from concourse import bacc
from concourse.bass_utils import run_bass_kernel_spmd
from concourse.masks import make_identity

F32 = mybir.dt.float32
F32R = mybir.dt.float32r
BF16 = mybir.dt.bfloat16
I16 = mybir.dt.int16
TABLE_DT = BF16  # dtype of the gathered feature tables (BF16 or F32R)

NCORES = 8
P = 128
SEL_BATCH = 8  # chunks per batched is_equal
GMAX = 8  # max chunks (128 idxs each) per dma_gather call


def _derive_cfg(n_nodes, f_in, f_mid, f_out):
    shard = n_nodes // NCORES
    assert shard * NCORES == n_nodes
    groups = -(-shard // P)
    sp = groups * P  # padded shard rows
    trows = NCORES * sp  # padded table rows
    nwin = -(-trows // 32768)
    wrow = -(-trows // nwin)
    wrow = -(-wrow // P) * P  # multiple of 128
    assert wrow <= 32768
    return dict(
        n=n_nodes, f_in=f_in, f1=f_mid, f2=f_out,
        shard=shard, groups=groups, sp=sp, trows=trows,
        nwin=nwin, wrow=wrow,
    )


# ----------------------------------------------------------------- host prep

def _host_prep(x, edge_index, cfg):
    n = cfg["n"]
    shard, groups, sp = cfg["shard"], cfg["groups"], cfg["sp"]
    nwin, wrow = cfg["nwin"], cfg["wrow"]

    row = np.asarray(edge_index[0], dtype=np.int64)
    col = np.asarray(edge_index[1], dtype=np.int64)
    loops = np.arange(n, dtype=np.int64)
    rows = np.concatenate([row, loops]).astype(np.int32)
    cols = np.concatenate([col, loops]).astype(np.int32)

    deg = np.bincount(rows, minlength=n).astype(np.float64)
    dinv = np.where(deg > 0, 1.0 / np.sqrt(deg), 0.0).astype(np.float32)

    # padded table row of a source node
    trow = (cols // shard) * sp + (cols % shard)
    kc = rows // shard                       # dest core
    kg = (rows % shard) // P                 # dest group within core
    kw = trow // wrow                        # source window
    key = (kc.astype(np.int64) * groups + kg) * nwin + kw
    order = np.argsort(key, kind="stable")
    rows_s = rows[order]
    trow_s = trow[order]
    key_s = key[order]

    counts = np.bincount(key_s, minlength=NCORES * groups * nwin)
    counts = counts.reshape(NCORES, groups, nwin)
    # compile-time chunk count per (group, window) slot: max over cores
    slot_chunks = -(-counts.max(axis=0) // P)  # [groups, nwin] ceil
    slot_edges = slot_chunks * P
    tc_total = int(slot_chunks.sum())  # total chunks per core

    # slot offsets (in edges) shared by all cores
    slot_off = np.zeros((groups, nwin), dtype=np.int64)
    off = 0
    for g in range(groups):
        for w in range(nwin):
            slot_off[g, w] = off
            off += int(slot_edges[g, w])
    pad_total = off  # == tc_total * P

    core_start = np.zeros(NCORES * groups * nwin + 1, dtype=np.int64)
    np.cumsum(counts.reshape(-1), out=core_start[1:])

    idx16_all, dl_all = [], []
    for c in range(NCORES):
        pc = np.zeros(pad_total, dtype=np.int16)        # gather idx (pad 0)
        pd = np.full(pad_total, 999.0, dtype=np.float32)  # dest_local (pad)
        for g in range(groups):
            for w in range(nwin):
                k = (c * groups + g) * nwin + w
                s, e = core_start[k], core_start[k + 1]
                cnt = e - s
                if cnt == 0:
                    continue
                o = slot_off[g, w]
                pc[o:o + cnt] = (trow_s[s:e] - w * wrow).astype(np.int16)
                pd[o:o + cnt] = (rows_s[s:e] - c * shard - g * P).astype(
                    np.float32)
        idx16 = pc.reshape(tc_total * 8, 16).T            # [16, tc*8]
        idx16 = np.ascontiguousarray(np.tile(idx16, (8, 1)))  # [128, tc*8]
        dl = np.ascontiguousarray(pd.reshape(tc_total, P).T)  # [128, tc]
        idx16_all.append(idx16)
        dl_all.append(dl)

    # per-core padded x shard + dinv tiles
    x = np.asarray(x, dtype=np.float32)
    x_sh, dinv_sh = [], []
    for c in range(NCORES):
        xs = np.zeros((sp, cfg["f_in"]), dtype=np.float32)
        xs[:shard] = x[c * shard:(c + 1) * shard]
        dv = np.zeros(sp, dtype=np.float32)
        dv[:shard] = dinv[c * shard:(c + 1) * shard]
        x_sh.append(xs)
        dinv_sh.append(np.ascontiguousarray(dv.reshape(groups, P).T))
    return dict(
        idx16=idx16_all, dl=dl_all, x_sh=x_sh, dinv_sh=dinv_sh,
        slot_chunks=slot_chunks, tc_total=tc_total,
    )


# ------------------------------------------------------------ device program

def _build_program(cfg, slot_chunks, with_bias, stage=99, fake_cc=False, reps=1, agg_mode='full'):
    """slot_chunks: [groups, nwin] int array of chunk counts per slot.

    stage (debug): 1=dense1 only, 2=+AG1, 3=+agg1, 4=+dense2, 5=+AG2,
    6=+agg2, 99=full.
    """
    f_in, f1, f2 = cfg["f_in"], cfg["f1"], cfg["f2"]
    groups, sp, trows = cfg["groups"], cfg["sp"], cfg["trows"]
    nwin, wrow = cfg["nwin"], cfg["wrow"]
    tc_total = int(slot_chunks.sum())
    cmax = int(slot_chunks.max())
    # per-group chunk totals
    gch = slot_chunks.sum(axis=1)  # [groups]

    nc = bacc.Bacc("TRN2", target_bir_lowering=False, debug=False,
                   num_devices=NCORES)

    # I/O
    x_in = nc.dram_tensor("x_sh", [sp, f_in], F32, kind="ExternalInput")
    dinv_in = nc.dram_tensor("dinv_sh", [P, groups], F32,
                             kind="ExternalInput")
    idx_in = nc.dram_tensor("idx16", [P, tc_total * 8], I16,
                            kind="ExternalInput")
    dl_in = nc.dram_tensor("dl", [P, tc_total], F32, kind="ExternalInput")
    w1_in = nc.dram_tensor("W1", [f_in, f1], F32, kind="ExternalInput")
    w2_in = nc.dram_tensor("W2", [f1, f2], F32, kind="ExternalInput")
    wmu_in = nc.dram_tensor("Wmu", [f2, f2], F32, kind="ExternalInput")
    wlv_in = nc.dram_tensor("Wlv", [f2, f2], F32, kind="ExternalInput")
    if with_bias:
        b1_in = nc.dram_tensor("b1t", [P, f1], F32, kind="ExternalInput")
        b2_in = nc.dram_tensor("b2t", [P, f2], F32, kind="ExternalInput")
        bmu_in = nc.dram_tensor("bmut", [P, f2], F32, kind="ExternalInput")
        blv_in = nc.dram_tensor("blvt", [P, f2], F32, kind="ExternalInput")
    out_mu = nc.dram_tensor("out_mu", [sp, f2], F32, kind="ExternalOutput")
    out_lv = nc.dram_tensor("out_lv", [sp, f2], F32, kind="ExternalOutput")

    # internal DRAM
    l1shard = nc.dram_tensor("l1shard", [sp, f1], TABLE_DT)
    l1table = nc.dram_tensor("l1table", [trows, f1], TABLE_DT,
                              addr_space="Shared")
    z1shard = nc.dram_tensor("z1shard", [sp, f1], F32)
    x2shard = nc.dram_tensor("x2shard", [sp, f2], TABLE_DT)
    l2table = nc.dram_tensor("l2table", [trows, f2], TABLE_DT,
                              addr_space="Shared")
    z2shard = nc.dram_tensor("z2shard", [sp, f2], F32)

    rg = [list(range(NCORES))]

    from contextlib import ExitStack
    with tile.TileContext(nc) as tc, ExitStack() as es:
        wpool = es.enter_context(tc.tile_pool(name="wpool", bufs=1))
        xpool = es.enter_context(tc.tile_pool(name="xpool", bufs=3))
        gpool = es.enter_context(tc.tile_pool(name="gpool", bufs=4))
        spool = es.enter_context(tc.tile_pool(name="spool", bufs=4))
        mpool = es.enter_context(tc.tile_pool(name="mpool", bufs=2))
        opool = es.enter_context(tc.tile_pool(name="opool", bufs=3))
        pp_t = es.enter_context(tc.tile_pool(name="pp_t", bufs=2, space="PSUM"))
        pp_a = es.enter_context(tc.tile_pool(name="pp_a", bufs=2, space="PSUM"))

        # ---- resident constants
        identity = wpool.tile([P, P], F32)
        make_identity(nc, identity[:])
        iota = wpool.tile([P, P], F32)
        nc.gpsimd.iota(iota[:], pattern=[[1, P]], base=0, channel_multiplier=0,
                       allow_small_or_imprecise_dtypes=True)
        dinv_t = wpool.tile([P, groups], F32)
        nc.sync.dma_start(out=dinv_t[:], in_=dinv_in[:, :])

        w1_t = wpool.tile([P, 2, f1], F32, tag="w1")
        for h in range(2):
            nc.sync.dma_start(out=w1_t[:, h, :],
                              in_=w1_in[h * P:(h + 1) * P, :])
        w2_t = wpool.tile([P, 2, f2], F32, tag="w2")
        for h in range(2):
            nc.sync.dma_start(out=w2_t[:, h, :],
                              in_=w2_in[h * P:(h + 1) * P, :])
        wmu_t = wpool.tile([P, f2], F32, tag="wmu")
        nc.sync.dma_start(out=wmu_t[:], in_=wmu_in[:, :])
        wlv_t = wpool.tile([P, f2], F32, tag="wlv")
        nc.sync.dma_start(out=wlv_t[:], in_=wlv_in[:, :])
        if with_bias:
            b1_t = wpool.tile([P, f1], F32, tag="b1")
            nc.sync.dma_start(out=b1_t[:], in_=b1_in[:, :])
            b2_t = wpool.tile([P, f2], F32, tag="b2")
            nc.sync.dma_start(out=b2_t[:], in_=b2_in[:, :])
            bmu_t = wpool.tile([P, f2], F32, tag="bmu")
            nc.sync.dma_start(out=bmu_t[:], in_=bmu_in[:, :])
            blv_t = wpool.tile([P, f2], F32, tag="blv")
            nc.sync.dma_start(out=blv_t[:], in_=blv_in[:, :])

        def dense_phase(src_dram, fin, fout, w_tile, dst_dram, tag):
            """dst = dinv * (src @ W); src node-major [sp, fin]."""
            kt = fin // P
            for g in range(groups):
                xt = xpool.tile([P, fin], F32, tag=f"{tag}_x")
                nc.sync.dma_start(out=xt[:],
                                  in_=src_dram[g * P:(g + 1) * P, :])
                xT = xpool.tile([P, kt, P], F32, tag=f"{tag}_xT")
                for h in range(kt):
                    pt = pp_t.tile([P, P], F32, tag="tp")
                    nc.tensor.transpose(out=pt[:], in_=xt[:, h * P:(h + 1) * P],
                                        identity=identity[:])
                    nc.vector.tensor_copy(out=xT[:, h, :], in_=pt[:])
                acc = pp_a.tile([P, fout], F32, tag=f"acc{fout}")
                for h in range(kt):
                    nc.tensor.matmul(out=acc[:], lhsT=xT[:, h, :],
                                     rhs=w_tile[:, h, :] if kt > 1
                                     else w_tile[:],
                                     start=(h == 0), stop=(h == kt - 1))
                ot = opool.tile([P, fout], TABLE_DT, tag=f"{tag}_o")
                nc.scalar.activation(ot[:], acc[:],
                                     mybir.ActivationFunctionType.Copy,
                                     scale=dinv_t[:, g:g + 1])
                nc.sync.dma_start(out=dst_dram[g * P:(g + 1) * P, :],
                                  in_=ot[:])

        def agg_phase(table_dram, f, dst_dram, relu, bias_t, tag):
            """dst[g] = act(dinv * sum_sel(gather(table))) + bias."""
            for g in range(groups):
                g_chunks = int(gch[g])
                if g_chunks == 0:
                    continue
                goff = int(slot_chunks[:g].sum())  # chunk offset of group
                # group metadata
                idx_t = mpool.tile([P, cmaxg * 8], I16, tag=f"{tag}_idx")
                nc.sync.dma_start(
                    out=idx_t[:, :g_chunks * 8],
                    in_=idx_in[:, goff * 8:(goff + g_chunks) * 8])
                dl_t = mpool.tile([P, cmaxg], F32, tag=f"{tag}_dl")
                nc.sync.dma_start(out=dl_t[:, :g_chunks],
                                  in_=dl_in[:, goff:goff + g_chunks])

                acc = pp_a.tile([P, f], F32, tag=f"acc{f}")
                kdone = 0  # chunks done within group
                loc = 0    # chunk offset within group metadata
                for w in range(nwin):
                    cgw = int(slot_chunks[g, w])
                    if cgw == 0:
                        continue
                    for k0 in range(0, cgw, GMAX):
                        gc = min(GMAX, cgw - k0)
                        gt = gpool.tile([P, GMAX, f], TABLE_DT, tag=f"{tag}_g")
                        if agg_mode != "compute_only":
                            nc.gpsimd.dma_gather(
                                gt[:, :gc, :],
                                table_dram[w * wrow:(w + 1) * wrow, :],
                                idx_t[:, (loc + k0) * 8:(loc + k0 + gc) * 8],
                                num_idxs=gc * P, num_idxs_reg=gc * P,
                                elem_size=f, single_packet=False,
                            )
                        if agg_mode == "gather_only":
                            loc_unused = 0  # no compute in this mode
                            kdone += gc
                            continue
                        sel = spool.tile([P, GMAX * P], TABLE_DT,
                                         tag=f"{tag}_sel")
                        nc.vector.tensor_tensor(
                            out=sel[:, :gc * P].rearrange(
                                "p (b c) -> p b c", c=P),
                            in0=dl_t[:, loc + k0:loc + k0 + gc].unsqueeze(2)
                                .to_broadcast([P, gc, P]),
                            in1=iota[:].unsqueeze(1).to_broadcast([P, gc, P]),
                            op=mybir.AluOpType.is_equal,
                        )
                        for k in range(gc):
                            nc.tensor.matmul(
                                out=acc[:],
                                lhsT=sel[:, k * P:(k + 1) * P],
                                rhs=gt[:, k, :],
                                start=(kdone == 0),
                                stop=(kdone == g_chunks - 1),
                            )
                            kdone += 1
                    loc += cgw
                if agg_mode == "gather_only":
                    continue
                ot = opool.tile([P, f], F32, tag=f"{tag}_o")
                if bias_t is None:
                    nc.vector.tensor_scalar_mul(ot[:], acc[:],
                                                dinv_t[:, g:g + 1])
                    if relu:
                        nc.vector.tensor_scalar_max(ot[:], ot[:], 0.0)
                else:
                    nc.vector.tensor_scalar_mul(ot[:], acc[:],
                                                dinv_t[:, g:g + 1])
                    nc.vector.tensor_tensor(out=ot[:], in0=ot[:],
                                            in1=bias_t[:],
                                            op=mybir.AluOpType.add)
                    if relu:
                        nc.scalar.activation(
                            ot[:], ot[:], mybir.ActivationFunctionType.Relu)
                nc.sync.dma_start(out=dst_dram[g * P:(g + 1) * P, :],
                                  in_=ot[:])

        cmaxg = int(gch.max())

        for _rep in range(reps):
            def dump_and_stop(src, f):
                """Debug early-exit: copy src rows into out_mu and finish."""
                for g in range(groups):
                    t = opool.tile([P, f2], F32, tag="dump")
                    nc.sync.dma_start(out=t[:], in_=src[g * P:(g + 1) * P, :f2])
                    nc.sync.dma_start(out=out_mu[g * P:(g + 1) * P, :], in_=t[:])
                    t2 = opool.tile([P, f2], F32, tag="dump")
                    nc.gpsimd.memset(t2[:], 0.0)
                    nc.sync.dma_start(out=out_lv[g * P:(g + 1) * P, :], in_=t2[:])

            # ---- layer 1
            dense_phase(x_in, f_in, f1, w1_t, l1shard, "d1")
            if stage == 1:
                dump_and_stop(l1shard, f1)
            if stage >= 2:
                if fake_cc:
                    for cc in range(NCORES):
                        nc.sync.dma_start(
                            out=l1table[cc * sp:(cc + 1) * sp, :],
                            in_=l1shard[:, :])
                else:
                    nc.gpsimd.collective_compute(
                        "AllGather", mybir.AluOpType.bypass, replica_groups=rg,
                        ins=[l1shard.ap().opt()], outs=[l1table.ap().opt()])
                if stage == 2:
                    dump_and_stop(l1table, f1)
            if stage >= 3:
                agg_phase(l1table, f1, z1shard, True,
                          b1_t if with_bias else None, "a1")
                if stage == 3:
                    dump_and_stop(z1shard, f1)

            # ---- layer 2
            if stage >= 4:
                dense_phase(z1shard, f1, f2, w2_t, x2shard, "d2")
                if stage == 4:
                    dump_and_stop(x2shard, f2)
            if stage >= 5:
                if fake_cc:
                    for cc in range(NCORES):
                        nc.sync.dma_start(
                            out=l2table[cc * sp:(cc + 1) * sp, :],
                            in_=x2shard[:, :])
                else:
                    nc.gpsimd.collective_compute(
                        "AllGather", mybir.AluOpType.bypass, replica_groups=rg,
                        ins=[x2shard.ap().opt()], outs=[l2table.ap().opt()])
                if stage == 5:
                    dump_and_stop(l2table, f2)
            if stage >= 6:
                agg_phase(l2table, f2, z2shard, False,
                          b2_t if with_bias else None, "a2")
                if stage == 6:
                    dump_and_stop(z2shard, f2)

            # ---- heads
            for g in range(groups if stage >= 7 else 0):
                zt = xpool.tile([P, f2], F32, tag="h_z")
                nc.sync.dma_start(out=zt[:], in_=z2shard[g * P:(g + 1) * P, :])
                pt = pp_t.tile([P, P], F32, tag="tp")
                nc.tensor.transpose(out=pt[:], in_=zt[:], identity=identity[:])
                zT = xpool.tile([P, P], F32, tag="h_zT")
                nc.vector.tensor_copy(out=zT[:], in_=pt[:])
                for w_t, b_t, dst in (
                    (wmu_t, bmu_t if with_bias else None, out_mu),
                    (wlv_t, blv_t if with_bias else None, out_lv),
                ):
                    acch = pp_a.tile([P, f2], F32, tag=f"acc{f2}")
                    nc.tensor.matmul(out=acch[:], lhsT=zT[:], rhs=w_t[:],
                                     start=True, stop=True)
                    oh = opool.tile([P, f2], F32, tag="h_o")
                    if b_t is None:
                        nc.scalar.activation(oh[:], acch[:],
                                             mybir.ActivationFunctionType.Copy)
                    else:
                        nc.vector.tensor_tensor(out=oh[:], in0=acch[:],
                                                in1=b_t[:],
                                                op=mybir.AluOpType.add)
                    nc.sync.dma_start(out=dst[g * P:(g + 1) * P, :], in_=oh[:])

    nc.compile()
    return nc


# ------------------------------------------------------------------- driver

_CACHE = {}
_RUNNERS = {}


def _get_runner(nc, key):
    """Cached jitted shard_map callable over the 8 cores for program `nc`."""
    if key in _RUNNERS:
        return _RUNNERS[key]
    import jax
    from jax.sharding import Mesh, PartitionSpec
    from jax.experimental.shard_map import shard_map
    from concourse import bass2jax

    bass2jax.install_neuronx_cc_hook()
    partition_name = (nc.partition_id_tensor.name
                      if nc.partition_id_tensor else None)
    in_names, out_names, out_avals, zero_shapes = [], [], [], []
    for alloc in nc.m.functions[0].allocations:
        if not isinstance(alloc, mybir.MemoryLocationSet):
            continue
        name = alloc.memorylocations[0].name
        if alloc.kind == "ExternalInput":
            if name != partition_name:
                in_names.append(name)
        elif alloc.kind == "ExternalOutput":
            shape = tuple(alloc.tensor_shape)
            dtype = mybir.dt.np(alloc.dtype)
            out_names.append(name)
            out_avals.append(jax.core.ShapedArray(shape, dtype))
            zero_shapes.append((shape, dtype))
    n_params = len(in_names)
    n_outs = len(out_avals)
    all_in_names = in_names + out_names + (
        [partition_name] if partition_name else [])

    def _body(*args):
        operands = list(args)
        if partition_name is not None:
            operands.append(bass2jax.partition_id_tensor())
        outs = bass2jax._bass_exec_p.bind(
            *operands, out_avals=tuple(out_avals),
            in_names=tuple(all_in_names), out_names=tuple(out_names),
            lowering_input_output_aliases=(), sim_require_finite=True,
            sim_require_nnan=True, nc=nc)
        return tuple(outs)

    devices = jax.devices()[:NCORES]
    mesh = Mesh(np.asarray(devices), ("core",))
    in_specs = (PartitionSpec("core"),) * (n_params + n_outs)
    out_specs = (PartitionSpec("core"),) * n_outs
    fn = jax.jit(
        shard_map(_body, mesh=mesh, in_specs=in_specs, out_specs=out_specs,
                  check_rep=False),
        keep_unused=True)
    r = dict(fn=fn, in_names=in_names, out_names=out_names,
             out_avals=out_avals, zero_shapes=zero_shapes)
    _RUNNERS[key] = r
    return r


def _run(nc, key, in_maps):
    r = _get_runner(nc, key)
    concat_in = [
        np.concatenate([np.asarray(in_maps[c][n]) for c in range(NCORES)],
                       axis=0)
        for n in r["in_names"]]
    concat_zeros = [np.zeros((NCORES * s[0], *s[1:]), d)
                    for s, d in r["zero_shapes"]]
    out = r["fn"](*concat_in, *concat_zeros)
    results = [
        {name: np.asarray(out[i]).reshape(NCORES, *r["out_avals"][i].shape)[c]
         for i, name in enumerate(r["out_names"])}
        for c in range(NCORES)]
    return results


def _get_program(cfg, slot_chunks, with_bias, stage=99):
    key = (tuple(sorted(cfg.items())), slot_chunks.tobytes(), with_bias, stage)
    if key not in _CACHE:
        _CACHE[key] = _build_program(cfg, slot_chunks, with_bias, stage)
    return _CACHE[key]


def kernel(x, edge_index, W1, b1, W2, b2, Wmu, bmu, Wlv, blv):
    n, f_in = x.shape
    f1 = W1.shape[1]
    f2 = W2.shape[1]
    cfg = _derive_cfg(n, f_in, f1, f2)
    prep = _host_prep(x, edge_index, cfg)
    with_bias = not (
        np.all(b1 == 0) and np.all(b2 == 0)
        and np.all(bmu == 0) and np.all(blv == 0))
    pkey = (tuple(sorted(cfg.items())), prep["slot_chunks"].tobytes(),
            with_bias, 99)
    nc = _get_program(cfg, prep["slot_chunks"], with_bias)

    in_maps = []
    for c in range(NCORES):
        m = {
            "x_sh": prep["x_sh"][c],
            "dinv_sh": prep["dinv_sh"][c],
            "idx16": prep["idx16"][c],
            "dl": prep["dl"][c],
            "W1": np.asarray(W1, np.float32),
            "W2": np.asarray(W2, np.float32),
            "Wmu": np.asarray(Wmu, np.float32),
            "Wlv": np.asarray(Wlv, np.float32),
        }
        if with_bias:
            m["b1t"] = np.tile(np.asarray(b1, np.float32), (P, 1))
            m["b2t"] = np.tile(np.asarray(b2, np.float32), (P, 1))
            m["bmut"] = np.tile(np.asarray(bmu, np.float32), (P, 1))
            m["blvt"] = np.tile(np.asarray(blv, np.float32), (P, 1))
        in_maps.append(m)

    try:
        results = _run(nc, pkey, in_maps)
    except Exception:
        results = run_bass_kernel_spmd(
            nc, in_maps, core_ids=list(range(NCORES))).results
    shard = cfg["shard"]
    mu = np.concatenate(
        [results[c]["out_mu"][:shard] for c in range(NCORES)], axis=0)
    lv = np.concatenate(
        [results[c]["out_lv"][:shard] for c in range(NCORES)], axis=0)
    return (mu, lv)



# revision 40
# speedup vs baseline: 3.4571x; 3.4571x over previous
"""GCN encoder (2x GCNConv + mu/logvar heads) on 8 Trainium2 NeuronCores.

Strategy: shard destination nodes (and their incoming edges) across the 8
cores.  Each layer:
  1. dense matmul of the core's node shard by the (replicated) weight,
     pre-scaled by dinv = deg^-1/2   (fp32 on PE, activations PE-transposed)
  2. AllGather the scaled feature table so every core can gather any source row
  3. aggregation over the core's dest-sorted edges: dma_gather source rows
     (int16 windowed indices), build one-hot selection matrices with DVE
     is_equal against an iota row, and accumulate 128-edge x 128-dest
     selection matmuls (float32r) into a PSUM tile per 128-dest group
  4. epilogue on ACT: out = relu(dinv * psum) (+bias) for layer 1,
     out = dinv * psum (+bias) for layer 2.
mu/logvar heads are small dense matmuls on the aggregated shard.
"""

import sys

import numpy as np

try:
    import concourse.bass as bass  # noqa: F401
except ImportError:
    sys.path.insert(0, "/opt/trn_rl_repo")

import concourse.bass as bass
import concourse.mybir as mybir
import concourse.tile as tile
from concourse import bacc
from concourse.bass_utils import run_bass_kernel_spmd
from concourse.masks import make_identity

F32 = mybir.dt.float32
F32R = mybir.dt.float32r
BF16 = mybir.dt.bfloat16
I16 = mybir.dt.int16
I32 = mybir.dt.int32
TABLE_DT = BF16  # dtype of the gathered feature tables (BF16 or F32R)

NCORES = 8
P = 128
SEL_BATCH = 8  # chunks per batched is_equal
GMAX = 8  # max chunks (128 idxs each) per dma_gather call


def _derive_cfg(n_nodes, f_in, f_mid, f_out):
    shard = n_nodes // NCORES
    assert shard * NCORES == n_nodes
    groups = -(-shard // P)
    sp = groups * P  # padded shard rows
    trows = NCORES * sp  # padded table rows
    nwin = -(-trows // 32768)
    wrow = -(-trows // nwin)
    wrow = -(-wrow // P) * P  # multiple of 128
    assert wrow <= 32768
    return dict(
        n=n_nodes, f_in=f_in, f1=f_mid, f2=f_out,
        shard=shard, groups=groups, sp=sp, trows=trows,
        nwin=nwin, wrow=wrow,
    )


# ----------------------------------------------------------------- host prep

def _call_spans(slot_chunks, gmax):
    """Gather-call structure, shared by host prep and program build.

    Yields (g, w, k0, gc) in exact program order; one gather call covers
    chunks [k0, k0+gc) of slot (g, w).
    """
    groups, nwin = slot_chunks.shape
    for g in range(groups):
        for w in range(nwin):
            cgw = int(slot_chunks[g, w])
            if cgw == 0:
                continue
            for k0 in range(0, cgw, gmax):
                yield g, w, k0, min(gmax, cgw - k0)


def _host_prep(x, edge_index, cfg, gmax=None, sort_rows=True):
    n = cfg["n"]
    shard, groups, sp = cfg["shard"], cfg["groups"], cfg["sp"]
    nwin, wrow = cfg["nwin"], cfg["wrow"]

    row = np.asarray(edge_index[0], dtype=np.int64)
    col = np.asarray(edge_index[1], dtype=np.int64)
    # self-loops are NOT materialized as edges: their contribution is the
    # node's own (pre-scaled) table row, added in the aggregation epilogue.
    rows = row.astype(np.int32)
    cols = col.astype(np.int32)

    deg = (np.bincount(rows, minlength=n) + 1).astype(np.float64)
    dinv = (1.0 / np.sqrt(deg)).astype(np.float32)

    # padded table row of a source node
    trow = (cols // shard) * sp + (cols % shard)
    kc = rows // shard                       # dest core
    kg = (rows % shard) // P                 # dest group within core
    kw = trow // wrow                        # source window
    key = (kc.astype(np.int64) * groups + kg) * nwin + kw
    # sort by slot, then ascending table row within the slot (HBM locality)
    order = (np.lexsort((trow, key)) if sort_rows
             else np.argsort(key, kind="stable"))
    rows_s = rows[order]
    trow_s = trow[order]
    key_s = key[order]

    counts = np.bincount(key_s, minlength=NCORES * groups * nwin)
    counts = counts.reshape(NCORES, groups, nwin)
    # compile-time chunk count per (group, window) slot: max over cores
    slot_chunks = -(-counts.max(axis=0) // P)  # [groups, nwin] ceil
    slot_edges = slot_chunks * P
    tc_total = int(slot_chunks.sum())  # total chunks per core

    # slot offsets (in edges) shared by all cores
    slot_off = np.zeros((groups, nwin), dtype=np.int64)
    off = 0
    for g in range(groups):
        for w in range(nwin):
            slot_off[g, w] = off
            off += int(slot_edges[g, w])
    pad_total = off  # == tc_total * P

    core_start = np.zeros(NCORES * groups * nwin + 1, dtype=np.int64)
    np.cumsum(counts.reshape(-1), out=core_start[1:])

    gm = gmax if gmax is not None else GMAX
    spans = list(_call_spans(slot_chunks, gm))
    idx16_all, dl_all, gcnt_all = [], [], []
    for c in range(NCORES):
        pc = np.full(pad_total, -1, dtype=np.int16)     # gather idx (pad -1)
        pd = np.full(pad_total, 999.0, dtype=np.float32)  # dest_local (pad)
        ccount = np.zeros((groups, nwin), dtype=np.int64)
        for g in range(groups):
            for w in range(nwin):
                k = (c * groups + g) * nwin + w
                s, e = core_start[k], core_start[k + 1]
                cnt = e - s
                ccount[g, w] = cnt
                if cnt == 0:
                    continue
                o = slot_off[g, w]
                pc[o:o + cnt] = (trow_s[s:e] - w * wrow).astype(np.int16)
                pd[o:o + cnt] = (rows_s[s:e] - c * shard - g * P).astype(
                    np.float32)
        # per-call valid-idx counts; a call with zero valid idxs gets one
        # dummy valid idx (the gather ucode requires >=1 non-negative idx)
        gcnt = np.zeros(len(spans), dtype=np.int32)
        for ci, (g, w, k0, gc) in enumerate(spans):
            cnt = int(np.clip(ccount[g, w] - k0 * P, 0, gc * P))
            if ci < 4:
                # first call per gather-tile pool buffer gathers full width so
                # rows skipped by later short calls always hold finite stale
                # data (0*stale must not produce NaN in the sel matmul)
                o = slot_off[g, w] + k0 * P
                seg = pc[o:o + gc * P]
                seg[seg == -1] = 0
                cnt = gc * P
            elif cnt == 0:
                pc[slot_off[g, w] + k0 * P] = 0
                cnt = 1
            gcnt[ci] = cnt
        idx16 = pc.reshape(tc_total * 8, 16).T            # [16, tc*8]
        idx16 = np.ascontiguousarray(np.tile(idx16, (8, 1)))  # [128, tc*8]
        dl = np.ascontiguousarray(pd.reshape(tc_total, P).T)  # [128, tc]
        idx16_all.append(idx16)
        dl_all.append(dl)
        gcnt_all.append(gcnt.reshape(1, -1))

    # per-core padded x shard + dinv tiles
    x = np.asarray(x, dtype=np.float32)
    x_sh, dinv_sh = [], []
    for c in range(NCORES):
        xs = np.zeros((sp, cfg["f_in"]), dtype=np.float32)
        xs[:shard] = x[c * shard:(c + 1) * shard]
        dv = np.zeros(sp, dtype=np.float32)
        dv[:shard] = dinv[c * shard:(c + 1) * shard]
        x_sh.append(xs)
        dinv_sh.append(np.ascontiguousarray(dv.reshape(groups, P).T))
    return dict(
        idx16=idx16_all, dl=dl_all, x_sh=x_sh, dinv_sh=dinv_sh,
        slot_chunks=slot_chunks, tc_total=tc_total, gcnt=gcnt_all,
        ncalls=len(spans),
    )


# ------------------------------------------------------------ device program

def _build_program(cfg, slot_chunks, with_bias, stage=99, fake_cc=False, reps=1, agg_mode='full', gmax=None, nqueues=1, scratch=16384, t1dt=None, spkt=False, qbywin=False):
    """slot_chunks: [groups, nwin] int array of chunk counts per slot.

    stage (debug): 1=dense1 only, 2=+AG1, 3=+agg1, 4=+dense2, 5=+AG2,
    6=+agg2, 99=full.
    """
    f_in, f1, f2 = cfg["f_in"], cfg["f1"], cfg["f2"]
    groups, sp, trows = cfg["groups"], cfg["sp"], cfg["trows"]
    nwin, wrow = cfg["nwin"], cfg["wrow"]
    tc_total = int(slot_chunks.sum())
    cmax = int(slot_chunks.max())
    # per-group chunk totals
    gch = slot_chunks.sum(axis=1)  # [groups]
    GM = gmax if gmax is not None else GMAX
    qctr = [0]  # round-robin SWDGE queue counter
    T1 = t1dt if t1dt is not None else TABLE_DT

    nc = bacc.Bacc("TRN2", target_bir_lowering=False, debug=False,
                   num_devices=NCORES, num_swdge_queues=nqueues,
                   dynamic_dma_scratch_size=scratch)

    # I/O
    ncalls = sum(1 for _ in _call_spans(slot_chunks, GM))
    x_in = nc.dram_tensor("x_sh", [sp, f_in], F32, kind="ExternalInput")
    dinv_in = nc.dram_tensor("dinv_sh", [P, groups], F32,
                             kind="ExternalInput")
    idx_in = nc.dram_tensor("idx16", [P, tc_total * 8], I16,
                            kind="ExternalInput")
    dl_in = nc.dram_tensor("dl", [P, tc_total], F32, kind="ExternalInput")
    gcnt_in = nc.dram_tensor("gcnt", [1, ncalls], I32, kind="ExternalInput")
    w1_in = nc.dram_tensor("W1", [f_in, f1], F32, kind="ExternalInput")
    w2_in = nc.dram_tensor("W2", [f1, f2], F32, kind="ExternalInput")
    wmu_in = nc.dram_tensor("Wmu", [f2, f2], F32, kind="ExternalInput")
    wlv_in = nc.dram_tensor("Wlv", [f2, f2], F32, kind="ExternalInput")
    if with_bias:
        b1_in = nc.dram_tensor("b1t", [P, f1], F32, kind="ExternalInput")
        b2_in = nc.dram_tensor("b2t", [P, f2], F32, kind="ExternalInput")
        bmu_in = nc.dram_tensor("bmut", [P, f2], F32, kind="ExternalInput")
        blv_in = nc.dram_tensor("blvt", [P, f2], F32, kind="ExternalInput")
    out_mu = nc.dram_tensor("out_mu", [sp, f2], F32, kind="ExternalOutput")
    out_lv = nc.dram_tensor("out_lv", [sp, f2], F32, kind="ExternalOutput")

    # internal DRAM
    l1shard = nc.dram_tensor("l1shard", [sp, f1], T1)
    l1table = nc.dram_tensor("l1table", [trows, f1], T1,
                              addr_space="Shared")
    z1shard = nc.dram_tensor("z1shard", [sp, f1], F32)
    x2shard = nc.dram_tensor("x2shard", [sp, f2], TABLE_DT)
    l2table = nc.dram_tensor("l2table", [trows, f2], TABLE_DT,
                              addr_space="Shared")
    z2shard = nc.dram_tensor("z2shard", [sp, f2], F32)

    rg = [list(range(NCORES))]

    from contextlib import ExitStack
    with tile.TileContext(nc) as tc, ExitStack() as es:
        wpool = es.enter_context(tc.tile_pool(name="wpool", bufs=1))
        xpool = es.enter_context(tc.tile_pool(name="xpool", bufs=3))
        gpool = es.enter_context(tc.tile_pool(name="gpool", bufs=4))
        spool = es.enter_context(tc.tile_pool(name="spool", bufs=4))
        mpool = es.enter_context(tc.tile_pool(name="mpool", bufs=2))
        opool = es.enter_context(tc.tile_pool(name="opool", bufs=3))
        pp_t = es.enter_context(tc.tile_pool(name="pp_t", bufs=2, space="PSUM"))
        pp_a = es.enter_context(tc.tile_pool(name="pp_a", bufs=2, space="PSUM"))

        # ---- resident constants
        identity = wpool.tile([P, P], F32)
        make_identity(nc, identity[:])
        iota = wpool.tile([P, P], F32)
        nc.gpsimd.iota(iota[:], pattern=[[1, P]], base=0, channel_multiplier=0,
                       allow_small_or_imprecise_dtypes=True)
        dinv_t = wpool.tile([P, groups], F32)
        nc.sync.dma_start(out=dinv_t[:], in_=dinv_in[:, :])
        cnt_t = wpool.tile([1, ncalls], I32, tag="gcnt")
        nc.sync.dma_start(out=cnt_t[:], in_=gcnt_in[:, :])
        greg = nc.gpsimd.alloc_register("gcnt_reg")

        w1_t = wpool.tile([P, 2, f1], F32, tag="w1")
        for h in range(2):
            nc.sync.dma_start(out=w1_t[:, h, :],
                              in_=w1_in[h * P:(h + 1) * P, :])
        w2_t = wpool.tile([P, 2, f2], F32, tag="w2")
        for h in range(2):
            nc.sync.dma_start(out=w2_t[:, h, :],
                              in_=w2_in[h * P:(h + 1) * P, :])
        wmu_t = wpool.tile([P, f2], F32, tag="wmu")
        nc.sync.dma_start(out=wmu_t[:], in_=wmu_in[:, :])
        wlv_t = wpool.tile([P, f2], F32, tag="wlv")
        nc.sync.dma_start(out=wlv_t[:], in_=wlv_in[:, :])
        if with_bias:
            b1_t = wpool.tile([P, f1], F32, tag="b1")
            nc.sync.dma_start(out=b1_t[:], in_=b1_in[:, :])
            b2_t = wpool.tile([P, f2], F32, tag="b2")
            nc.sync.dma_start(out=b2_t[:], in_=b2_in[:, :])
            bmu_t = wpool.tile([P, f2], F32, tag="bmu")
            nc.sync.dma_start(out=bmu_t[:], in_=bmu_in[:, :])
            blv_t = wpool.tile([P, f2], F32, tag="blv")
            nc.sync.dma_start(out=blv_t[:], in_=blv_in[:, :])

        def dense_phase(src_dram, fin, fout, w_tile, dst_dram, tag,
                        out_dt=TABLE_DT):
            """dst = dinv * (src @ W); src node-major [sp, fin]."""
            kt = fin // P
            for g in range(groups):
                xt = xpool.tile([P, fin], F32, tag=f"{tag}_x")
                nc.sync.dma_start(out=xt[:],
                                  in_=src_dram[g * P:(g + 1) * P, :])
                xT = xpool.tile([P, kt, P], F32, tag=f"{tag}_xT")
                for h in range(kt):
                    pt = pp_t.tile([P, P], F32, tag="tp")
                    nc.tensor.transpose(out=pt[:], in_=xt[:, h * P:(h + 1) * P],
                                        identity=identity[:])
                    nc.vector.tensor_copy(out=xT[:, h, :], in_=pt[:])
                acc = pp_a.tile([P, fout], F32, tag=f"acc{fout}")
                for h in range(kt):
                    nc.tensor.matmul(out=acc[:], lhsT=xT[:, h, :],
                                     rhs=w_tile[:, h, :] if kt > 1
                                     else w_tile[:],
                                     start=(h == 0), stop=(h == kt - 1))
                ot = opool.tile([P, fout], out_dt, tag=f"{tag}_o")
                nc.scalar.activation(ot[:], acc[:],
                                     mybir.ActivationFunctionType.Copy,
                                     scale=dinv_t[:, g:g + 1])
                nc.sync.dma_start(out=dst_dram[g * P:(g + 1) * P, :],
                                  in_=ot[:])

        def agg_phase(table_dram, f, dst_dram, relu, bias_t, tag,
                      table_dt=TABLE_DT, own_dram=None):
            """dst[g] = act(dinv * (sum_sel(gather(table)) + own)) + bias.

            own_dram: this core's pre-scaled shard (self-loop contribution).
            """
            aci = [0]  # call ordinal, matches _call_spans order
            for g in range(groups):
                g_chunks = int(gch[g])
                goff = int(slot_chunks[:g].sum())  # chunk offset of group
                # group metadata
                if g_chunks:
                    idx_t = mpool.tile([P, cmaxg * 8], I16, tag=f"{tag}_idx")
                    nc.sync.dma_start(
                        out=idx_t[:, :g_chunks * 8],
                        in_=idx_in[:, goff * 8:(goff + g_chunks) * 8])
                    dl_t = mpool.tile([P, cmaxg], F32, tag=f"{tag}_dl")
                    nc.sync.dma_start(out=dl_t[:, :g_chunks],
                                      in_=dl_in[:, goff:goff + g_chunks])
                    acc = pp_a.tile([P, f], F32, tag=f"acc{f}")
                kdone = 0  # chunks done within group
                loc = 0    # chunk offset within group metadata
                for w in range(nwin):
                    cgw = int(slot_chunks[g, w])
                    if cgw == 0:
                        continue
                    for k0 in range(0, cgw, GM):
                        gc = min(GM, cgw - k0)
                        gt = gpool.tile([P, GM, f], table_dt, tag=f"{tag}_g")
                        if agg_mode != "compute_only":
                            ci = aci[0]
                            nc.gpsimd.reg_load(greg, cnt_t[0:1, ci:ci + 1])
                            nc.gpsimd.dma_gather(
                                gt[:, :gc, :],
                                table_dram[w * wrow:(w + 1) * wrow, :],
                                idx_t[:, (loc + k0) * 8:(loc + k0 + gc) * 8],
                                num_idxs=gc * P, num_idxs_reg=greg,
                                elem_size=f, single_packet=spkt,
                                queue_num=(w % nqueues) if qbywin
                                else qctr[0] % nqueues,
                            )
                            qctr[0] += 1
                        aci[0] += 1
                        if agg_mode == "gather_only":
                            loc_unused = 0  # no compute in this mode
                            kdone += gc
                            continue
                        sel = spool.tile([P, GM * P], table_dt,
                                         tag=f"{tag}_sel")
                        nc.vector.tensor_tensor(
                            out=sel[:, :gc * P].rearrange(
                                "p (b c) -> p b c", c=P),
                            in0=dl_t[:, loc + k0:loc + k0 + gc].unsqueeze(2)
                                .to_broadcast([P, gc, P]),
                            in1=iota[:].unsqueeze(1).to_broadcast([P, gc, P]),
                            op=mybir.AluOpType.is_equal,
                        )
                        for k in range(gc):
                            nc.tensor.matmul(
                                out=acc[:],
                                lhsT=sel[:, k * P:(k + 1) * P],
                                rhs=gt[:, k, :],
                                start=(kdone == 0),
                                stop=(kdone == g_chunks - 1),
                            )
                            kdone += 1
                    loc += cgw
                if agg_mode == "gather_only":
                    continue
                ot = opool.tile([P, f], F32, tag=f"{tag}_o")
                own = opool.tile([P, f], table_dt, tag=f"{tag}_own")
                nc.sync.dma_start(out=own[:],
                                  in_=own_dram[g * P:(g + 1) * P, :])
                if g_chunks:
                    nc.vector.tensor_tensor(out=ot[:], in0=acc[:],
                                            in1=own[:],
                                            op=mybir.AluOpType.add)
                    nc.vector.tensor_scalar_mul(ot[:], ot[:],
                                                dinv_t[:, g:g + 1])
                else:
                    nc.vector.tensor_scalar_mul(ot[:], own[:],
                                                dinv_t[:, g:g + 1])
                if bias_t is not None:
                    nc.vector.tensor_tensor(out=ot[:], in0=ot[:],
                                            in1=bias_t[:],
                                            op=mybir.AluOpType.add)
                if relu:
                    nc.vector.tensor_scalar_max(ot[:], ot[:], 0.0)
                nc.sync.dma_start(out=dst_dram[g * P:(g + 1) * P, :],
                                  in_=ot[:])

        cmaxg = int(gch.max())

        for _rep in range(reps):
            def dump_and_stop(src, f):
                """Debug early-exit: copy src rows into out_mu and finish."""
                for g in range(groups):
                    ts = opool.tile([P, f2], src.dtype, tag="dump_s")
                    nc.sync.dma_start(out=ts[:],
                                      in_=src[g * P:(g + 1) * P, :f2])
                    t = opool.tile([P, f2], F32, tag="dump")
                    nc.vector.tensor_copy(out=t[:], in_=ts[:])
                    nc.sync.dma_start(out=out_mu[g * P:(g + 1) * P, :], in_=t[:])
                    t2 = opool.tile([P, f2], F32, tag="dump")
                    nc.gpsimd.memset(t2[:], 0.0)
                    nc.sync.dma_start(out=out_lv[g * P:(g + 1) * P, :], in_=t2[:])

            # ---- layer 1
            dense_phase(x_in, f_in, f1, w1_t, l1shard, "d1", out_dt=T1)
            if stage == 1:
                dump_and_stop(l1shard, f1)
            if stage >= 2:
                if fake_cc:
                    for cc in range(NCORES):
                        nc.sync.dma_start(
                            out=l1table[cc * sp:(cc + 1) * sp, :],
                            in_=l1shard[:, :])
                else:
                    nc.gpsimd.collective_compute(
                        "AllGather", mybir.AluOpType.bypass, replica_groups=rg,
                        ins=[l1shard.ap().opt()], outs=[l1table.ap().opt()])
                if stage == 2:
                    dump_and_stop(l1table, f1)
            if stage >= 3:
                agg_phase(l1table, f1, z1shard, True,
                          b1_t if with_bias else None, "a1", table_dt=T1,
                          own_dram=l1shard)
                if stage == 3:
                    dump_and_stop(z1shard, f1)

            # ---- layer 2
            if stage >= 4:
                dense_phase(z1shard, f1, f2, w2_t, x2shard, "d2")
                if stage == 4:
                    dump_and_stop(x2shard, f2)
            if stage >= 5:
                if fake_cc:
                    for cc in range(NCORES):
                        nc.sync.dma_start(
                            out=l2table[cc * sp:(cc + 1) * sp, :],
                            in_=x2shard[:, :])
                else:
                    nc.gpsimd.collective_compute(
                        "AllGather", mybir.AluOpType.bypass, replica_groups=rg,
                        ins=[x2shard.ap().opt()], outs=[l2table.ap().opt()])
                if stage == 5:
                    dump_and_stop(l2table, f2)
            if stage >= 6:
                agg_phase(l2table, f2, z2shard, False,
                          b2_t if with_bias else None, "a2",
                          own_dram=x2shard)
                if stage == 6:
                    dump_and_stop(z2shard, f2)

            # ---- heads
            for g in range(groups if stage >= 7 else 0):
                zt = xpool.tile([P, f2], F32, tag="h_z")
                nc.sync.dma_start(out=zt[:], in_=z2shard[g * P:(g + 1) * P, :])
                pt = pp_t.tile([P, P], F32, tag="tp")
                nc.tensor.transpose(out=pt[:], in_=zt[:], identity=identity[:])
                zT = xpool.tile([P, P], F32, tag="h_zT")
                nc.vector.tensor_copy(out=zT[:], in_=pt[:])
                for w_t, b_t, dst in (
                    (wmu_t, bmu_t if with_bias else None, out_mu),
                    (wlv_t, blv_t if with_bias else None, out_lv),
                ):
                    acch = pp_a.tile([P, f2], F32, tag=f"acc{f2}")
                    nc.tensor.matmul(out=acch[:], lhsT=zT[:], rhs=w_t[:],
                                     start=True, stop=True)
                    oh = opool.tile([P, f2], F32, tag="h_o")
                    if b_t is None:
                        nc.scalar.activation(oh[:], acch[:],
                                             mybir.ActivationFunctionType.Copy)
                    else:
                        nc.vector.tensor_tensor(out=oh[:], in0=acch[:],
                                                in1=b_t[:],
                                                op=mybir.AluOpType.add)
                    nc.sync.dma_start(out=dst[g * P:(g + 1) * P, :], in_=oh[:])

    nc.compile()
    return nc


# ------------------------------------------------------------------- driver

_CACHE = {}
_RUNNERS = {}


def _get_runner(nc, key):
    """Cached jitted shard_map callable over the 8 cores for program `nc`."""
    if key in _RUNNERS:
        return _RUNNERS[key]
    import jax
    from jax.sharding import Mesh, PartitionSpec
    from jax.experimental.shard_map import shard_map
    from concourse import bass2jax

    bass2jax.install_neuronx_cc_hook()
    partition_name = (nc.partition_id_tensor.name
                      if nc.partition_id_tensor else None)
    in_names, out_names, out_avals, zero_shapes = [], [], [], []
    for alloc in nc.m.functions[0].allocations:
        if not isinstance(alloc, mybir.MemoryLocationSet):
            continue
        name = alloc.memorylocations[0].name
        if alloc.kind == "ExternalInput":
            if name != partition_name:
                in_names.append(name)
        elif alloc.kind == "ExternalOutput":
            shape = tuple(alloc.tensor_shape)
            dtype = mybir.dt.np(alloc.dtype)
            out_names.append(name)
            out_avals.append(jax.core.ShapedArray(shape, dtype))
            zero_shapes.append((shape, dtype))
    n_params = len(in_names)
    n_outs = len(out_avals)
    all_in_names = in_names + out_names + (
        [partition_name] if partition_name else [])

    def _body(*args):
        operands = list(args)
        if partition_name is not None:
            operands.append(bass2jax.partition_id_tensor())
        outs = bass2jax._bass_exec_p.bind(
            *operands, out_avals=tuple(out_avals),
            in_names=tuple(all_in_names), out_names=tuple(out_names),
            lowering_input_output_aliases=(), sim_require_finite=True,
            sim_require_nnan=True, nc=nc)
        return tuple(outs)

    devices = jax.devices()[:NCORES]
    mesh = Mesh(np.asarray(devices), ("core",))
    in_specs = (PartitionSpec("core"),) * (n_params + n_outs)
    out_specs = (PartitionSpec("core"),) * n_outs
    fn = jax.jit(
        shard_map(_body, mesh=mesh, in_specs=in_specs, out_specs=out_specs,
                  check_rep=False),
        keep_unused=True)
    r = dict(fn=fn, in_names=in_names, out_names=out_names,
             out_avals=out_avals, zero_shapes=zero_shapes)
    _RUNNERS[key] = r
    return r


def _run(nc, key, in_maps):
    r = _get_runner(nc, key)
    concat_in = [
        np.concatenate([np.asarray(in_maps[c][n]) for c in range(NCORES)],
                       axis=0)
        for n in r["in_names"]]
    concat_zeros = [np.zeros((NCORES * s[0], *s[1:]), d)
                    for s, d in r["zero_shapes"]]
    out = r["fn"](*concat_in, *concat_zeros)
    results = [
        {name: np.asarray(out[i]).reshape(NCORES, *r["out_avals"][i].shape)[c]
         for i, name in enumerate(r["out_names"])}
        for c in range(NCORES)]
    return results


BEST_OPTS = dict(nqueues=4, gmax=32)


def _get_program(cfg, slot_chunks, with_bias, stage=99, **opts):
    opts = {**BEST_OPTS, **opts}
    key = (tuple(sorted(cfg.items())), slot_chunks.tobytes(), with_bias,
           stage, tuple(sorted((k, str(v)) for k, v in opts.items())))
    if key not in _CACHE:
        _CACHE[key] = _build_program(cfg, slot_chunks, with_bias, stage,
                                     **opts)
    return _CACHE[key]


def kernel(x, edge_index, W1, b1, W2, b2, Wmu, bmu, Wlv, blv):
    n, f_in = x.shape
    f1 = W1.shape[1]
    f2 = W2.shape[1]
    cfg = _derive_cfg(n, f_in, f1, f2)
    prep = _host_prep(x, edge_index, cfg, gmax=BEST_OPTS["gmax"])
    with_bias = not (
        np.all(b1 == 0) and np.all(b2 == 0)
        and np.all(bmu == 0) and np.all(blv == 0))
    pkey = (tuple(sorted(cfg.items())), prep["slot_chunks"].tobytes(),
            with_bias, 99)
    nc = _get_program(cfg, prep["slot_chunks"], with_bias)

    in_maps = []
    for c in range(NCORES):
        m = {
            "x_sh": prep["x_sh"][c],
            "dinv_sh": prep["dinv_sh"][c],
            "idx16": prep["idx16"][c],
            "dl": prep["dl"][c],
            "gcnt": prep["gcnt"][c],
            "W1": np.asarray(W1, np.float32),
            "W2": np.asarray(W2, np.float32),
            "Wmu": np.asarray(Wmu, np.float32),
            "Wlv": np.asarray(Wlv, np.float32),
        }
        if with_bias:
            m["b1t"] = np.tile(np.asarray(b1, np.float32), (P, 1))
            m["b2t"] = np.tile(np.asarray(b2, np.float32), (P, 1))
            m["bmut"] = np.tile(np.asarray(bmu, np.float32), (P, 1))
            m["blvt"] = np.tile(np.asarray(blv, np.float32), (P, 1))
        in_maps.append(m)

    try:
        results = _run(nc, pkey, in_maps)
    except Exception:
        results = run_bass_kernel_spmd(
            nc, in_maps, core_ids=list(range(NCORES))).results
    shard = cfg["shard"]
    mu = np.concatenate(
        [results[c]["out_mu"][:shard] for c in range(NCORES)], axis=0)
    lv = np.concatenate(
        [results[c]["out_lv"][:shard] for c in range(NCORES)], axis=0)
    return (mu, lv)

